# revision 33
# baseline (speedup 1.0000x reference)
"""Windowed sparse point-transformer layer on 8 Trainium2 NeuronCores.

Strategy (spec sharding_hint): windows are independent, so the host
scatters voxels into dense per-window buffers (the "all-to-all keyed by
window id" done as a host-side relayout), shards 900->928 windows across
8 cores (116 each), and each core runs a dense windowed attention+FFN
Bass kernel. Weights are replicated. The host gathers per-window outputs
back to the sparse voxel list.

Device kernel per window (T=128 slots, C=48 channels, 8 heads x 6 dim):
  - q/k projections produce CHANNEL-major qT/kT with heads padded onto
    32-partition strips so the 8 score matmuls use PE row tiling
    (K=8 per head: 6 data lanes + a ones/mask lane that applies the
    -1e5 padding mask during the matmul itself; bias is folded in via
    the input's ones row).
  - softmax (q-major): exp on ACT with fused row-sum accumulation,
    reciprocal + per-head normalize on DVE (bf16), attn transposed on
    PE, ctx matmuls against bf16 v accumulate fp32 in PSUM.
  - residual 1: x.T is PE-transposed straight into the same PSUM
    accumulation group as the output projection (ctx @ Wo + bo).
  - FFN batched across a 4-window chunk (N=512) in float32r so the PE
    runs at full rate; b1/b2 ride the activation bias port during PSUM
    eviction. LayerNorms run token-major on DVE/ACT (bn_stats/bn_aggr).

PSUM is 8 banks; tile tags are assigned so the live set maps onto 8
bank-sized slots (s0-s3 shared by projections/scores/FFN, at, misc,
cx, y1).
"""

import numpy as np
import ml_dtypes
from contextlib import ExitStack

GX, GY, GZ = 120, 120, 8
WX, WY, WZ = 8, 8, 2
T = 128
C = 48
H = 8
HD = 6
FF = 256
N = 80000
NW = (GX // WX) * (GY // WY) * (GZ // WZ)  # 900
NCORES = 8
WPC = 116  # windows per core (900 padded to 928 = 8*116)
CHUNK = 4  # windows per projection/FFN batch (N=512 tokens)
MASKVAL = -1e5

_CACHE = {}


def _build_bass(wpc, reps=1, ablate=()):
    import concourse.bass as bass
    import concourse.tile as tile
    import concourse.mybir as mybir
    from concourse import bacc
    from concourse.masks import make_identity

    f32 = mybir.dt.float32
    f32r = mybir.dt.float32r
    bf16 = mybir.dt.bfloat16
    AF = mybir.ActivationFunctionType
    ALU = mybir.AluOpType

    nc = bacc.Bacc("TRN2", target_bir_lowering=False, debug=False)
    ntok = wpc * T

    xta = nc.dram_tensor("xta", [50, ntok], f32r, kind="ExternalInput")
    wqa = nc.dram_tensor("wqa", [50, 128], f32r, kind="ExternalInput")
    wqb = nc.dram_tensor("wqb", [50, 128], f32r, kind="ExternalInput")
    wka = nc.dram_tensor("wka", [50, 128], f32r, kind="ExternalInput")
    wkb = nc.dram_tensor("wkb", [50, 128], f32r, kind="ExternalInput")
    wv = nc.dram_tensor("wv", [50, 48], f32r, kind="ExternalInput")
    wopa = nc.dram_tensor("wopa", [128, 48], bf16, kind="ExternalInput")
    wopb = nc.dram_tensor("wopb", [128, 48], bf16, kind="ExternalInput")
    boc = nc.dram_tensor("boc", [48, 1], f32, kind="ExternalInput")
    w1 = nc.dram_tensor("w1", [48, 256], f32r, kind="ExternalInput")
    w2 = nc.dram_tensor("w2", [256, 48], f32r, kind="ExternalInput")
    b1c = nc.dram_tensor("b1c", [128, 2], f32, kind="ExternalInput")
    b2c = nc.dram_tensor("b2c", [48, 1], f32, kind="ExternalInput")
    lnc = nc.dram_tensor("lnc", [4, 48], f32, kind="ExternalInput")
    out = nc.dram_tensor("out", [wpc, T, C], f32, kind="ExternalOutput")

    nchunk = wpc // CHUNK
    assert wpc % CHUNK == 0

    def _bn(col, n):
        return bass.AP(tensor=col.tensor, offset=col.offset,
                       ap=[col.ap[0], [0, n]])

    def _b(ap3, n=None):
        # broadcast a [128, k] AP along a new trailing free dim of size 48
        return bass.AP(tensor=ap3.tensor, offset=ap3.offset,
                       ap=list(ap3.ap) + [[0, 48]])

    def ln_apply(pool, y_ap, g_ap, b_ap, eps_s, out_dtype, tagp):
        """LayerNorm over the free dim (C=48) of token-major y_ap."""
        mv = pool.tile([128, 2], f32, tag=f"{tagp}mv", name=f"{tagp}mv")
        st6 = pool.tile([128, 6], f32, tag=f"{tagp}st", name=f"{tagp}st")
        nc.vector.bn_stats(out=st6[:], in_=y_ap)
        nc.vector.bn_aggr(out=mv[:], in_=st6[:])
        rstd = pool.tile([128, 1], f32, tag=f"{tagp}rs", name=f"{tagp}rs")
        nc.scalar.activation(
            out=rstd[:], in_=mv[:, 1:2], func=AF.Sqrt, bias=eps_s[:]
        )
        nc.vector.reciprocal(rstd[:], rstd[:])
        yc = pool.tile([128, 48], f32, tag=f"{tagp}yc", name=f"{tagp}yc")
        nc.vector.tensor_scalar_sub(yc[:], y_ap, mv[:, 0:1])
        ho = pool.tile([128, 48], out_dtype, tag=f"{tagp}ho", name=f"{tagp}ho")
        nc.vector.scalar_tensor_tensor(
            out=ho[:], in0=yc[:], scalar=rstd[:], in1=g_ap,
            op0=ALU.mult, op1=ALU.mult,
        )
        nc.vector.tensor_add(ho[:], ho[:], b_ap)
        return ho

    with tile.TileContext(nc) as tc, ExitStack() as ctx:
        singles = ctx.enter_context(tc.tile_pool(name="singles", bufs=1))

        xta_s = singles.tile([50, ntok], f32r)
        nc.sync.dma_start(out=xta_s[:], in_=xta.ap())
        w_tiles = {}
        for nm, hnd, shp in (
            ("wqa", wqa, [50, 128]), ("wqb", wqb, [50, 128]),
            ("wka", wka, [50, 128]), ("wkb", wkb, [50, 128]),
            ("wv", wv, [50, 48]), ("w1", w1, [48, 256]),
        ):
            t = singles.tile(shp, f32r, name=nm)
            nc.sync.dma_start(out=t[:], in_=hnd.ap())
            w_tiles[nm] = t
        wopa_s = singles.tile([128, 48], bf16)
        nc.sync.dma_start(out=wopa_s[:], in_=wopa.ap())
        wopb_s = singles.tile([128, 48], bf16)
        nc.sync.dma_start(out=wopb_s[:], in_=wopb.ap())
        bo_s = singles.tile([48, 1], f32)
        nc.sync.dma_start(out=bo_s[:], in_=boc.ap())
        w2_s = singles.tile([128, 2, 48], f32r)
        nc.sync.dma_start(
            out=w2_s[:], in_=w2.ap().rearrange("(two p) n -> p two n", two=2)
        )
        b1_s = singles.tile([128, 2], f32)
        nc.sync.dma_start(out=b1_s[:], in_=b1c.ap())
        b2_s = singles.tile([48, 1], f32)
        nc.sync.dma_start(out=b2_s[:], in_=b2c.ap())
        ln_s = singles.tile([128, 4, 48], f32)
        lnap = lnc.ap()
        ln_bcast = bass.AP(
            tensor=lnap.tensor, offset=lnap.offset, ap=[[0, 128]] + list(lnap.ap)
        )
        nc.sync.dma_start(out=ln_s[:], in_=ln_bcast)
        eps_s = singles.tile([128, 1], f32)
        nc.vector.memset(eps_s[:], 1e-5)
        idb = singles.tile([128, 128], bf16)
        make_identity(nc, idb[:])
        idtmp = singles.tile([128, 128], f32)
        make_identity(nc, idtmp[:])
        idf = singles.tile([128, 128], f32r)
        nc.vector.tensor_copy(out=idf[:], in_=idtmp[:])

        ps = ctx.enter_context(tc.tile_pool(name="ps", bufs=1, space="PSUM"))
        pss = ctx.enter_context(tc.tile_pool(name="pss", bufs=1, space="PSUM"))
        # ctx accumulators: heads live at partitions 32j..32j+6; the gap rows
        # must be exactly zero (o-projection multiplies them by zero weights,
        # and stale PSUM could be non-finite), so zero them once up front.
        cxp_t = pss.tile([128, 2, 128], f32, name="cxp_t")
        nc.vector.memset(cxp_t[:], 0.0)
        cxp = [cxp_t[:, 0, :], cxp_t[:, 1, :]]
        qk_sb = ctx.enter_context(tc.tile_pool(name="qk_sb", bufs=3))
        sm_sb = ctx.enter_context(tc.tile_pool(name="sm_sb", bufs=3))
        ln_sb = ctx.enter_context(tc.tile_pool(name="ln_sb", bufs=3))
        ff_sb = ctx.enter_context(tc.tile_pool(name="ff_sb", bufs=3))
        out_sb = ctx.enter_context(tc.tile_pool(name="out_sb", bufs=3))

        AX = mybir.AxisListType

        def ln_batch(y_ap, g_ap, b_ap, pfx, ch, out_dtype=None):
            """Chunk-batched LayerNorm: y_ap is [128, CHUNK, 48] (PSUM ok)."""
            od = out_dtype if out_dtype is not None else f32r
            P = ln_sb
            ssum = P.tile([128, CHUNK], f32, tag=f"{pfx}sum", name=f"{pfx}sum")
            nc.vector.tensor_reduce(out=ssum[:], in_=y_ap, axis=AX.X, op=ALU.add)
            sq = P.tile([128, CHUNK, 48], f32, tag=f"{pfx}sq", name=f"{pfx}sq")
            nc.scalar.square(out=sq[:], in_=y_ap)
            sqs = P.tile([128, CHUNK], f32, tag=f"{pfx}sqs", name=f"{pfx}sqs")
            nc.vector.tensor_reduce(out=sqs[:], in_=sq[:], axis=AX.X, op=ALU.add)
            mean = P.tile([128, CHUNK], f32, tag=f"{pfx}mn", name=f"{pfx}mn")
            nc.vector.tensor_scalar_mul(mean[:], ssum[:], 1.0 / 48.0)
            msq = P.tile([128, CHUNK], f32, tag=f"{pfx}msq", name=f"{pfx}msq")
            nc.vector.tensor_mul(msq[:], mean[:], mean[:])
            var = P.tile([128, CHUNK], f32, tag=f"{pfx}var", name=f"{pfx}var")
            nc.vector.scalar_tensor_tensor(
                out=var[:], in0=sqs[:], scalar=1.0 / 48.0, in1=msq[:],
                op0=ALU.mult, op1=ALU.subtract,
            )
            rstd = P.tile([128, CHUNK], f32, tag=f"{pfx}rs", name=f"{pfx}rs")
            nc.scalar.activation(
                out=rstd[:], in_=var[:], func=AF.Sqrt, bias=eps_s[:]
            )
            nc.vector.reciprocal(rstd[:], rstd[:])
            t = P.tile([128, CHUNK, 48], f32, tag=f"{pfx}t", name=f"{pfx}t")
            nc.vector.tensor_sub(t[:], y_ap, _b(mean[:]))
            nc.vector.tensor_mul(t[:], t[:], _b(rstd[:]))
            gv = bass.AP(tensor=g_ap.tensor, offset=g_ap.offset,
                         ap=[g_ap.ap[0], [0, CHUNK], g_ap.ap[1]])
            bv = bass.AP(tensor=b_ap.tensor, offset=b_ap.offset,
                         ap=[b_ap.ap[0], [0, CHUNK], b_ap.ap[1]])
            h = P.tile([128, CHUNK, 48], od, tag=f"{pfx}h", name=f"{pfx}h")
            nc.vector.tensor_mul(h[:], t[:], gv)
            nc.vector.tensor_add(h[:], h[:], bv)
            return h

        rep_cm = tc.For_i(0, reps, 1) if reps > 1 else None
        if rep_cm is not None:
            rep_cm.__enter__()
        for ch in range(nchunk):
            t0 = ch * CHUNK * T
            NTK = CHUNK * T
            xsl = xta_s[:, t0 : t0 + NTK]

            projs = {}
            for nm, wt, tag in (
                ("qa", "wqa", "s0"), ("qb", "wqb", "s1"),
                ("ka", "wka", "s2"), ("kb", "wkb", "s3"),
            ):
                p = ps.tile([128, NTK], f32, tag=tag, name=f"p{nm}")
                nc.tensor.matmul(p[:], w_tiles[wt][:], xsl)
                projs[nm] = p
            vt_p = ps.tile([48, NTK], f32, tag="misc", name="vt_p")
            nc.tensor.matmul(vt_p[:], w_tiles["wv"][:], xsl)

            qkev = {}
            for i, nm in enumerate(("qa", "qb", "ka", "kb")):
                t = qk_sb.tile([128, NTK], bf16, tag=nm, name=f"s{nm}")
                if i % 2 == 0:
                    nc.vector.tensor_copy(out=t[:], in_=projs[nm][:])
                else:
                    nc.scalar.copy(out=t[:], in_=projs[nm][:])
                qkev[nm] = t
            vt_s = qk_sb.tile([48, NTK], bf16, tag="vt", name="vt_s")
            nc.scalar.copy(out=vt_s[:], in_=vt_p[:])

            h1t_s = ff_sb.tile([48, NTK], f32r, tag="h1t", name="h1t_s")

            y1_p = ps.tile([128, CHUNK, 48], f32r, tag="y1", name="y1_p")
            for wi in range(CHUNK):
                w0 = wi * T

                v_ps = ps.tile([128, 48], bf16, tag="misc", name="v_ps")
                nc.tensor.transpose(v_ps[:], vt_s[:, w0 : w0 + T], idb[0:48, 0:48])
                v_s = sm_sb.tile([128, 48], bf16, tag="v", name="v_s")
                nc.vector.tensor_copy(out=v_s[:], in_=v_ps[:])

                scp = [
                    ps.tile([128, 2, 128], f32, tag=f"s{i}", name=f"scp{i}")
                    for i in range(4)
                ]
                if "scores" in ablate:
                    pass
                else:
                 for rnd, (qs, ks) in enumerate(
                    ((qkev["qa"], qkev["ka"]), (qkev["qb"], qkev["kb"]))
                ):
                    for i in range(4):
                        nc.tensor.matmul(
                            scp[i][:, rnd, :],
                            qs[32 * i : 32 * i + 8, w0 : w0 + T],
                            ks[32 * i : 32 * i + 8, w0 : w0 + T],
                            tile_position=(32 * i, 0),
                        )

                # attn layout: [128, strip i, round r, 128]; head h = 4*r + i
                attn_s = sm_sb.tile([128, 4, 2, 128], bf16, tag="attn", name="attn_s")
                for i in range(4 if "scores" not in ablate else 0):
                    nc.scalar.activation(
                        out=attn_s[:, i, :, :], in_=scp[i][:, :, :], func=AF.Exp
                    )
                if "attn_tail" in ablate:
                    ot_s = sm_sb.tile([48, 128], f32r, tag="ots", name="ot_s")
                    nc.scalar.copy(out=ot_s[:], in_=xsl[0:48, w0 : w0 + T])
                rsum = sm_sb.tile([128, 4, 2], f32, tag="rsum", name="rsum")
                rrec = sm_sb.tile([128, 4, 2], bf16, tag="rrec", name="rrec")
                for i in range(4):
                    nc.vector.tensor_reduce(
                        out=rsum[:, i, :], in_=attn_s[:, i, :, :], axis=AX.X,
                        op=ALU.add,
                    )
                    with nc.allow_low_precision(reason="softmax denom in bf16"):
                        nc.vector.reciprocal(rrec[:, i, :], rsum[:, i, :])
                    ri = rrec[:, i, :]
                    rrec_b = bass.AP(
                        tensor=ri.tensor, offset=ri.offset,
                        ap=list(ri.ap) + [[0, 128]],
                    )
                    nc.vector.tensor_mul(
                        attn_s[:, i, :, :], attn_s[:, i, :, :], rrec_b
                    )

                atp = ps.tile([128, 8, 128], bf16, tag="misc", name="atp")
                for h in range(8 if "attn_tail" not in ablate else 0):
                    nc.tensor.transpose(
                        atp[:, h, :], attn_s[:, h % 4, h // 4, :], idb[:]
                    )
                atn_s = sm_sb.tile([128, 8, 128], bf16, tag="atn", name="atn_s")
                if "attn_tail" not in ablate:
                    nc.vector.tensor_copy(out=atn_s[:, 0:4, :], in_=atp[:, 0:4, :])
                    nc.scalar.copy(out=atn_s[:, 4:8, :], in_=atp[:, 4:8, :])

                # ctx col-tiled: round r holds heads 4r+j at partitions 32j
                for h in range(8 if "attn_tail" not in ablate else 0):
                    r, j = divmod(h, 4)
                    nc.tensor.matmul(
                        cxp[r][32 * j : 32 * j + 6, :],
                        v_s[:, 6 * h : 6 * h + 6],
                        atn_s[:, h, :],
                        tile_position=(0, 32 * j),
                    )
                cxs = []
                for r in range(2 if "attn_tail" not in ablate else 0):
                    t = sm_sb.tile([128, 128], bf16, tag=f"cxs{r}", name=f"cxs{r}")
                    if r == 0:
                        nc.vector.tensor_copy(out=t[:], in_=cxp[r])
                    else:
                        nc.scalar.copy(out=t[:], in_=cxp[r])
                    cxs.append(t)

                # oT = sum_r WoPad_r.T @ cxs_r  (+bo on eviction)
                if "attn_tail" not in ablate:
                    ot_p = ps.tile([48, 128], f32, tag="late", name="ot_p")
                    nc.tensor.matmul(
                        ot_p[:], wopa_s[:], cxs[0][:], start=True, stop=False
                    )
                    nc.tensor.matmul(
                        ot_p[:], wopb_s[:], cxs[1][:], start=False, stop=True
                    )
                    ot_s = sm_sb.tile([48, 128], f32r, tag="ots", name="ot_s")
                    nc.scalar.activation(
                        out=ot_s[:], in_=ot_p[:], func=AF.Identity, bias=bo_s[:]
                    )

                # y1 = x + o via two transposes into one PSUM accum group
                y1w = y1_p[:, wi, :]
                nc.tensor.matmul(
                    y1w,
                    xsl[0:48, w0 : w0 + T],
                    idf[0:48, 0:48],
                    is_transpose=True,
                    start=True,
                    stop=False,
                )
                nc.tensor.matmul(
                    y1w,
                    ot_s[:],
                    idf[0:48, 0:48],
                    is_transpose=True,
                    start=False,
                    stop=True,
                )


            # batched LN1 over the whole chunk (PSUM input; every ln_batch
            # op reads it at most once per instruction)
            h1_s = ln_batch(y1_p[:].bitcast(f32), ln_s[:, 0, :], ln_s[:, 1, :],
                            "ln1", ch)
            for wi in range(CHUNK):
                w0 = wi * T
                h1t_p = ps.tile([48, 128], f32r, tag="late", name="h1t_p")
                nc.tensor.transpose(h1t_p[:], h1_s[:, wi, :], idf[:])
                nc.scalar.copy(out=h1t_s[:, w0 : w0 + T], in_=h1t_p[:])

            fr_s = []
            for half in range(2):
                ft_p = ps.tile([128, NTK], f32, tag=("late" if half == 0 else "misc"), name=f"ft{half}")
                nc.tensor.matmul(
                    ft_p[:], w_tiles["w1"][:, 128 * half : 128 * half + 128],
                    h1t_s[:],
                )
                fr = ff_sb.tile([128, NTK], f32r, tag=f"fr{half}", name=f"fr{half}")
                if half == 0:
                    nc.scalar.activation(
                        out=fr[:], in_=ft_p[:], func=AF.Relu,
                        bias=b1_s[:, half : half + 1],
                    )
                else:
                    nc.vector.tensor_scalar(
                        out=fr[:], in0=ft_p[:],
                        scalar1=b1_s[:, half : half + 1], scalar2=0.0,
                        op0=ALU.add, op1=ALU.max,
                    )
                fr_s.append(fr)
            y2t_p = ps.tile([48, NTK], f32, tag="y1", name="y2t_p")
            nc.tensor.matmul(
                y2t_p[:], w2_s[:, 0, :], fr_s[0][:], start=True, stop=False
            )
            nc.tensor.matmul(
                y2t_p[:], w2_s[:, 1, :], fr_s[1][:], start=False, stop=True
            )
            y2t_s = ff_sb.tile([48, NTK], f32r, tag="y2ts", name="y2t_s")
            nc.scalar.activation(
                out=y2t_s[:], in_=y2t_p[:], func=AF.Identity, bias=b2_s[:]
            )

            y2_p = ps.tile([128, CHUNK, 48], f32r, tag="y1", name="y2_p")
            for wi in range(CHUNK):
                w0 = wi * T
                nc.tensor.transpose(
                    y2_p[:, wi, :], y2t_s[:, w0 : w0 + T], idf[0:48, 0:48]
                )
            y3 = out_sb.tile([128, CHUNK, 48], f32, tag="y3", name="y3")
            nc.vector.tensor_add(
                y3[:], y2_p[:].bitcast(f32), h1_s[:].bitcast(f32)
            )
            h2 = ln_batch(y3[:], ln_s[:, 2, :], ln_s[:, 3, :], "ln2", ch, out_dtype=f32)
            if "dma" in ablate:
                continue
            nc.sync.dma_start(
                out=out.ap()[ch * CHUNK : (ch + 1) * CHUNK, :, :].rearrange(
                    "w t c -> t w c"
                ),
                in_=h2[:].bitcast(f32),
            )
        if rep_cm is not None:
            rep_cm.__exit__(None, None, None)

    nc.compile()
    return nc


def _prep_host(voxel_features, voxel_coords, Wq, bq, Wk, bk, Wv, bv, Wo, bo,
               ln1_g, ln1_b, W1, b1, W2, b2, ln2_g, ln2_b, wpc=WPC,
               ncores=NCORES):
    f32 = np.float32
    vc = np.asarray(voxel_coords)
    b, z, y, x = vc[:, 0], vc[:, 1], vc[:, 2], vc[:, 3]
    win = ((b * (GZ // WZ) + z // WZ) * (GY // WY) + y // WY) * (GX // WX) + x // WX
    slot = (z % WZ) * (WY * WX) + (y % WY) * WX + (x % WX)
    win = np.asarray(win, np.int64)
    slot = np.asarray(slot, np.int64)

    nwp = ncores * wpc
    xta = np.zeros((nwp, 50, T), f32)
    xta[:, 48, :] = 1.0
    xta[win, :48, slot] = np.asarray(voxel_features, f32)
    mask = np.full((nwp, T), MASKVAL, f32)
    occupied = np.zeros(nwp, bool)
    occupied[win] = True
    mask[~occupied] = 0.0
    mask[win, slot] = 0.0
    xta[:, 49, :] = mask

    s = f32(1.0 / np.sqrt(HD))
    Wq_s = np.asarray(Wq, f32) * s
    bq_s = np.asarray(bq, f32) * s
    bf = ml_dtypes.bfloat16

    def qk_pack(W, bvec, mask_lane):
        A = np.zeros((2, 50, 128), f32)
        for h in range(8):
            half, i = divmod(h, 4)
            A[half, :48, 32 * i : 32 * i + 6] = W[:, 6 * h : 6 * h + 6]
            A[half, 48, 32 * i : 32 * i + 6] = bvec[6 * h : 6 * h + 6]
            A[half, 49 if mask_lane else 48, 32 * i + 6] = 1.0
        return A[0], A[1]

    wqa_a, wqb_a = qk_pack(Wq_s, bq_s, mask_lane=False)
    wka_a, wkb_a = qk_pack(np.asarray(Wk, f32), np.asarray(bk, f32),
                           mask_lane=True)
    wv_a = np.zeros((50, 48), f32)
    wv_a[:48] = np.asarray(Wv, f32)
    wv_a[48] = np.asarray(bv, f32)
    wop = np.zeros((2, 128, 48), f32)
    for h in range(8):
        r, j = divmod(h, 4)
        wop[r, 32 * j : 32 * j + 6, :] = np.asarray(Wo, f32)[6 * h : 6 * h + 6, :]
    w1_a = np.ascontiguousarray(np.asarray(W1, f32))
    b1c_a = np.stack([np.asarray(b1, f32)[:128], np.asarray(b1, f32)[128:]], 1)
    b2c_a = np.asarray(b2, f32).reshape(48, 1)
    ln_a = np.stack([ln1_g, ln1_b, ln2_g, ln2_b]).astype(f32)

    weights = dict(
        wqa=wqa_a, wqb=wqb_a, wka=wka_a, wkb=wkb_a, wv=wv_a,
        wopa=wop[0].astype(bf), wopb=wop[1].astype(bf),
        boc=np.asarray(bo, f32).reshape(48, 1),
        w1=w1_a, w2=np.ascontiguousarray(np.asarray(W2, f32)),
        b1c=np.ascontiguousarray(b1c_a), b2c=b2c_a,
        lnc=np.ascontiguousarray(ln_a),
    )
    in_maps = []
    for c in range(ncores):
        m = dict(weights)
        sh = xta[c * wpc : (c + 1) * wpc]  # [wpc, 50, T]
        m["xta"] = np.ascontiguousarray(
            sh.transpose(1, 0, 2).reshape(50, wpc * T)
        )
        in_maps.append(m)
    return in_maps, win, slot


def kernel(**inputs):
    key = ("full", WPC)
    if key not in _CACHE:
        _CACHE[key] = _build_bass(WPC)
    nc = _CACHE[key]
    in_maps, win, slot = _prep_host(**inputs)
    from concourse import bass_utils

    r = bass_utils.run_bass_kernel_spmd(
        nc, in_maps, core_ids=list(range(NCORES))
    )
    full = np.concatenate([r.results[c]["out"] for c in range(NCORES)], 0)
    return full[win, slot].astype(np.float32)


# revision 35
# speedup vs baseline: 1.1288x; 1.1288x over previous
"""Windowed sparse point-transformer layer on 8 Trainium2 NeuronCores.

Strategy (spec sharding_hint): windows are independent, so the host
scatters voxels into dense per-window buffers (the "all-to-all keyed by
window id" done as a host-side relayout), shards 900->928 windows across
8 cores (116 each), and each core runs a dense windowed attention+FFN
Bass kernel. Weights are replicated. The host gathers per-window outputs
back to the sparse voxel list.

Device kernel per window (T=128 slots, C=48 channels, 8 heads x 6 dim):
  - q/k projections produce CHANNEL-major qT/kT with heads padded onto
    32-partition strips so the 8 score matmuls use PE row tiling
    (K=8 per head: 6 data lanes + a ones/mask lane that applies the
    -1e5 padding mask during the matmul itself; bias is folded in via
    the input's ones row).
  - softmax (q-major): exp on ACT with fused row-sum accumulation,
    reciprocal + per-head normalize on DVE (bf16), attn transposed on
    PE, ctx matmuls against bf16 v accumulate fp32 in PSUM.
  - residual 1: x.T is PE-transposed straight into the same PSUM
    accumulation group as the output projection (ctx @ Wo + bo).
  - FFN batched across a 4-window chunk (N=512) in float32r so the PE
    runs at full rate; b1/b2 ride the activation bias port during PSUM
    eviction. LayerNorms run token-major on DVE/ACT (bn_stats/bn_aggr).

PSUM is 8 banks; tile tags are assigned so the live set maps onto 8
bank-sized slots (s0-s3 shared by projections/scores/FFN, at, misc,
cx, y1).
"""

import numpy as np
import ml_dtypes
from contextlib import ExitStack

GX, GY, GZ = 120, 120, 8
WX, WY, WZ = 8, 8, 2
T = 128
C = 48
H = 8
HD = 6
FF = 256
N = 80000
NW = (GX // WX) * (GY // WY) * (GZ // WZ)  # 900
NCORES = 8
WPC = 116  # windows per core (900 padded to 928 = 8*116)
CHUNK = 4  # windows per projection/FFN batch (N=512 tokens)
MASKVAL = -1e5

_CACHE = {}


def _build_bass(wpc, reps=1, ablate=()):
    import concourse.bass as bass
    import concourse.tile as tile
    import concourse.mybir as mybir
    from concourse import bacc
    from concourse.masks import make_identity

    f32 = mybir.dt.float32
    f32r = mybir.dt.float32r
    bf16 = mybir.dt.bfloat16
    AF = mybir.ActivationFunctionType
    ALU = mybir.AluOpType

    nc = bacc.Bacc("TRN2", target_bir_lowering=False, debug=False)
    ntok = wpc * T

    xta = nc.dram_tensor("xta", [50, ntok], f32r, kind="ExternalInput")
    wqa = nc.dram_tensor("wqa", [50, 128], f32r, kind="ExternalInput")
    wqb = nc.dram_tensor("wqb", [50, 128], f32r, kind="ExternalInput")
    wka = nc.dram_tensor("wka", [50, 128], f32r, kind="ExternalInput")
    wkb = nc.dram_tensor("wkb", [50, 128], f32r, kind="ExternalInput")
    wv = nc.dram_tensor("wv", [50, 48], f32r, kind="ExternalInput")
    wopa = nc.dram_tensor("wopa", [128, 48], bf16, kind="ExternalInput")
    wopb = nc.dram_tensor("wopb", [128, 48], bf16, kind="ExternalInput")
    boc = nc.dram_tensor("boc", [48, 1], f32, kind="ExternalInput")
    w1 = nc.dram_tensor("w1", [48, 256], f32r, kind="ExternalInput")
    w2 = nc.dram_tensor("w2", [256, 48], f32r, kind="ExternalInput")
    b1c = nc.dram_tensor("b1c", [128, 2], f32, kind="ExternalInput")
    b2c = nc.dram_tensor("b2c", [48, 1], f32, kind="ExternalInput")
    lnc = nc.dram_tensor("lnc", [4, 48], f32, kind="ExternalInput")
    out = nc.dram_tensor("out", [wpc, T, C], f32, kind="ExternalOutput")

    nchunk = wpc // CHUNK
    assert wpc % CHUNK == 0

    def _bn(col, n):
        return bass.AP(tensor=col.tensor, offset=col.offset,
                       ap=[col.ap[0], [0, n]])

    def _b(ap3, n=None):
        # broadcast a [128, k] AP along a new trailing free dim of size 48
        return bass.AP(tensor=ap3.tensor, offset=ap3.offset,
                       ap=list(ap3.ap) + [[0, 48]])

    def ln_apply(pool, y_ap, g_ap, b_ap, eps_s, out_dtype, tagp):
        """LayerNorm over the free dim (C=48) of token-major y_ap."""
        mv = pool.tile([128, 2], f32, tag=f"{tagp}mv", name=f"{tagp}mv")
        st6 = pool.tile([128, 6], f32, tag=f"{tagp}st", name=f"{tagp}st")
        nc.vector.bn_stats(out=st6[:], in_=y_ap)
        nc.vector.bn_aggr(out=mv[:], in_=st6[:])
        rstd = pool.tile([128, 1], f32, tag=f"{tagp}rs", name=f"{tagp}rs")
        nc.scalar.activation(
            out=rstd[:], in_=mv[:, 1:2], func=AF.Sqrt, bias=eps_s[:]
        )
        nc.vector.reciprocal(rstd[:], rstd[:])
        yc = pool.tile([128, 48], f32, tag=f"{tagp}yc", name=f"{tagp}yc")
        nc.vector.tensor_scalar_sub(yc[:], y_ap, mv[:, 0:1])
        ho = pool.tile([128, 48], out_dtype, tag=f"{tagp}ho", name=f"{tagp}ho")
        nc.vector.scalar_tensor_tensor(
            out=ho[:], in0=yc[:], scalar=rstd[:], in1=g_ap,
            op0=ALU.mult, op1=ALU.mult,
        )
        nc.vector.tensor_add(ho[:], ho[:], b_ap)
        return ho

    with tile.TileContext(nc) as tc, ExitStack() as ctx:
        singles = ctx.enter_context(tc.tile_pool(name="singles", bufs=1))

        xta_s = singles.tile([50, ntok], f32r)
        nc.sync.dma_start(out=xta_s[:], in_=xta.ap())
        w_tiles = {}
        for nm, hnd, shp in (
            ("wqa", wqa, [50, 128]), ("wqb", wqb, [50, 128]),
            ("wka", wka, [50, 128]), ("wkb", wkb, [50, 128]),
            ("wv", wv, [50, 48]), ("w1", w1, [48, 256]),
        ):
            t = singles.tile(shp, f32r, name=nm)
            nc.sync.dma_start(out=t[:], in_=hnd.ap())
            w_tiles[nm] = t
        wopa_s = singles.tile([128, 48], bf16)
        nc.sync.dma_start(out=wopa_s[:], in_=wopa.ap())
        wopb_s = singles.tile([128, 48], bf16)
        nc.sync.dma_start(out=wopb_s[:], in_=wopb.ap())
        bo_s = singles.tile([48, 1], f32)
        nc.sync.dma_start(out=bo_s[:], in_=boc.ap())
        w2_s = singles.tile([128, 2, 48], f32r)
        nc.sync.dma_start(
            out=w2_s[:], in_=w2.ap().rearrange("(two p) n -> p two n", two=2)
        )
        b1_s = singles.tile([128, 2], f32)
        nc.sync.dma_start(out=b1_s[:], in_=b1c.ap())
        b2_s = singles.tile([48, 1], f32)
        nc.sync.dma_start(out=b2_s[:], in_=b2c.ap())
        ln_s = singles.tile([128, 4, 48], f32)
        lnap = lnc.ap()
        ln_bcast = bass.AP(
            tensor=lnap.tensor, offset=lnap.offset, ap=[[0, 128]] + list(lnap.ap)
        )
        nc.sync.dma_start(out=ln_s[:], in_=ln_bcast)
        eps_s = singles.tile([128, 1], f32)
        nc.vector.memset(eps_s[:], 1e-5)
        idb = singles.tile([128, 128], bf16)
        make_identity(nc, idb[:])
        idtmp = singles.tile([128, 128], f32)
        make_identity(nc, idtmp[:])
        idf = singles.tile([128, 128], f32r)
        nc.vector.tensor_copy(out=idf[:], in_=idtmp[:])

        ps = ctx.enter_context(tc.tile_pool(name="ps", bufs=1, space="PSUM"))
        pss = ctx.enter_context(tc.tile_pool(name="pss", bufs=1, space="PSUM"))
        # ctx accumulators: heads live at partitions 32j..32j+6; the gap rows
        # must be exactly zero (o-projection multiplies them by zero weights,
        # and stale PSUM could be non-finite), so zero them once up front.
        cxp_t = pss.tile([128, 2, 128], f32, name="cxp_t")
        nc.vector.memset(cxp_t[:], 0.0)
        cxp = [cxp_t[:, 0, :], cxp_t[:, 1, :]]
        qk_sb = ctx.enter_context(tc.tile_pool(name="qk_sb", bufs=3))
        sm_sb = ctx.enter_context(tc.tile_pool(name="sm_sb", bufs=3))
        ln_sb = ctx.enter_context(tc.tile_pool(name="ln_sb", bufs=3))
        ff_sb = ctx.enter_context(tc.tile_pool(name="ff_sb", bufs=3))
        out_sb = ctx.enter_context(tc.tile_pool(name="out_sb", bufs=3))

        AX = mybir.AxisListType

        def ln_batch(y_ap, g_ap, b_ap, pfx, ch, out_dtype=None):
            """Chunk-batched LayerNorm: y_ap is [128, CHUNK, 48] (PSUM ok)."""
            od = out_dtype if out_dtype is not None else f32r
            P = ln_sb
            ssum = P.tile([128, CHUNK], f32, tag=f"{pfx}sum", name=f"{pfx}sum")
            nc.vector.tensor_reduce(out=ssum[:], in_=y_ap, axis=AX.X, op=ALU.add)
            sq = P.tile([128, CHUNK, 48], f32, tag=f"{pfx}sq", name=f"{pfx}sq")
            nc.scalar.square(out=sq[:], in_=y_ap)
            sqs = P.tile([128, CHUNK], f32, tag=f"{pfx}sqs", name=f"{pfx}sqs")
            nc.vector.tensor_reduce(out=sqs[:], in_=sq[:], axis=AX.X, op=ALU.add)
            mean = P.tile([128, CHUNK], f32, tag=f"{pfx}mn", name=f"{pfx}mn")
            nc.vector.tensor_scalar_mul(mean[:], ssum[:], 1.0 / 48.0)
            msq = P.tile([128, CHUNK], f32, tag=f"{pfx}msq", name=f"{pfx}msq")
            nc.vector.tensor_mul(msq[:], mean[:], mean[:])
            var = P.tile([128, CHUNK], f32, tag=f"{pfx}var", name=f"{pfx}var")
            nc.vector.scalar_tensor_tensor(
                out=var[:], in0=sqs[:], scalar=1.0 / 48.0, in1=msq[:],
                op0=ALU.mult, op1=ALU.subtract,
            )
            rstd = P.tile([128, CHUNK], f32, tag=f"{pfx}rs", name=f"{pfx}rs")
            nc.scalar.activation(
                out=rstd[:], in_=var[:], func=AF.Sqrt, bias=eps_s[:]
            )
            nc.vector.reciprocal(rstd[:], rstd[:])
            t = P.tile([128, CHUNK, 48], f32, tag=f"{pfx}t", name=f"{pfx}t")
            nc.vector.tensor_sub(t[:], y_ap, _b(mean[:]))
            nc.vector.tensor_mul(t[:], t[:], _b(rstd[:]))
            gv = bass.AP(tensor=g_ap.tensor, offset=g_ap.offset,
                         ap=[g_ap.ap[0], [0, CHUNK], g_ap.ap[1]])
            bv = bass.AP(tensor=b_ap.tensor, offset=b_ap.offset,
                         ap=[b_ap.ap[0], [0, CHUNK], b_ap.ap[1]])
            h = P.tile([128, CHUNK, 48], od, tag=f"{pfx}h", name=f"{pfx}h")
            nc.vector.tensor_mul(h[:], t[:], gv)
            nc.vector.tensor_add(h[:], h[:], bv)
            return h

        rep_cm = tc.For_i(0, reps, 1) if reps > 1 else None
        if rep_cm is not None:
            rep_cm.__enter__()
        for ch in range(nchunk):
            t0 = ch * CHUNK * T
            NTK = CHUNK * T
            xsl = xta_s[:, t0 : t0 + NTK]

            projs = {}
            for nm, wt, tag in (
                ("qa", "wqa", "s0"), ("qb", "wqb", "s1"),
                ("ka", "wka", "s2"), ("kb", "wkb", "s3"),
            ):
                p = ps.tile([128, NTK], f32, tag=tag, name=f"p{nm}")
                nc.tensor.matmul(p[:], w_tiles[wt][:], xsl)
                projs[nm] = p
            vt_p = ps.tile([48, NTK], f32, tag="misc", name="vt_p")
            nc.tensor.matmul(vt_p[:], w_tiles["wv"][:], xsl)

            qkev = {}
            for i, nm in enumerate(("qa", "qb", "ka", "kb")):
                t = qk_sb.tile([128, NTK], bf16, tag=nm, name=f"s{nm}")
                if i % 2 == 0:
                    nc.vector.tensor_copy(out=t[:], in_=projs[nm][:])
                else:
                    nc.scalar.copy(out=t[:], in_=projs[nm][:])
                qkev[nm] = t
            vt_s = qk_sb.tile([48, NTK], bf16, tag="vt", name="vt_s")
            nc.scalar.copy(out=vt_s[:], in_=vt_p[:])

            h1t_s = ff_sb.tile([48, NTK], f32r, tag="h1t", name="h1t_s")

            y1_p = ps.tile([128, CHUNK, 48], f32r, tag="y1", name="y1_p")
            for wi in range(CHUNK):
                w0 = wi * T

                v_ps = ps.tile([128, 48], bf16, tag="misc", name="v_ps")
                nc.tensor.transpose(v_ps[:], vt_s[:, w0 : w0 + T], idb[0:48, 0:48])
                v_s = sm_sb.tile([128, 48], bf16, tag="v", name="v_s")
                nc.vector.tensor_copy(out=v_s[:], in_=v_ps[:])

                scp = [
                    ps.tile([128, 2, 128], f32, tag=f"s{i}", name=f"scp{i}")
                    for i in range(4)
                ]
                if "scores" not in ablate:
                    for rnd, (qs, ks) in enumerate(
                        ((qkev["qa"], qkev["ka"]), (qkev["qb"], qkev["kb"]))
                    ):
                        for i in range(4):
                            nc.tensor.matmul(
                                scp[i][:, rnd, :],
                                qs[32 * i : 32 * i + 8, w0 : w0 + T],
                                ks[32 * i : 32 * i + 8, w0 : w0 + T],
                                tile_position=(32 * i, 0),
                            )

                # attn layout: [128, strip i, round r, 128]; head h = 4*r + i
                attn_s = sm_sb.tile([128, 4, 2, 128], bf16, tag="attn", name="attn_s")
                for i in range(4 if "scores" not in ablate else 0):
                    nc.scalar.activation(
                        out=attn_s[:, i, :, :], in_=scp[i][:, :, :], func=AF.Exp
                    )
                if "attn_tail" in ablate:
                    ot_s = sm_sb.tile([48, 128], f32r, tag="ots", name="ot_s")
                    nc.scalar.copy(out=ot_s[:], in_=xsl[0:48, w0 : w0 + T])
                rsum = sm_sb.tile([128, 4, 2], f32, tag="rsum", name="rsum")
                rrec = sm_sb.tile([128, 4, 2], bf16, tag="rrec", name="rrec")
                for j in range(2):
                    sl = slice(2 * j, 2 * j + 2)
                    nc.vector.tensor_reduce(
                        out=rsum[:, sl, :], in_=attn_s[:, sl, :, :], axis=AX.X,
                        op=ALU.add,
                    )
                    with nc.allow_low_precision(reason="softmax denom in bf16"):
                        nc.vector.reciprocal(rrec[:, sl, :], rsum[:, sl, :])
                    rj = rrec[:, sl, :]
                    rrec_b = bass.AP(
                        tensor=rj.tensor, offset=rj.offset,
                        ap=list(rj.ap) + [[0, 128]],
                    )
                    nc.vector.tensor_mul(
                        attn_s[:, sl, :, :], attn_s[:, sl, :, :], rrec_b
                    )

                atp = ps.tile([128, 8, 128], bf16, tag="misc", name="atp")
                for h in range(8 if "attn_tail" not in ablate else 0):
                    nc.tensor.transpose(
                        atp[:, h, :], attn_s[:, h % 4, h // 4, :], idb[:]
                    )
                atn_s = sm_sb.tile([128, 8, 128], bf16, tag="atn", name="atn_s")
                if "attn_tail" not in ablate:
                    nc.vector.tensor_copy(out=atn_s[:, 0:4, :], in_=atp[:, 0:4, :])
                    nc.scalar.copy(out=atn_s[:, 4:8, :], in_=atp[:, 4:8, :])

                # ctx col-tiled: round r holds heads 4r+j at partitions 32j
                for h in range(8 if "attn_tail" not in ablate else 0):
                    r, j = divmod(h, 4)
                    nc.tensor.matmul(
                        cxp[r][32 * j : 32 * j + 6, :],
                        v_s[:, 6 * h : 6 * h + 6],
                        atn_s[:, h, :],
                        tile_position=(0, 32 * j),
                    )
                cxs = []
                for r in range(2 if "attn_tail" not in ablate else 0):
                    t = sm_sb.tile([128, 128], bf16, tag=f"cxs{r}", name=f"cxs{r}")
                    if r == 0:
                        nc.vector.tensor_copy(out=t[:], in_=cxp[r])
                    else:
                        nc.scalar.copy(out=t[:], in_=cxp[r])
                    cxs.append(t)

                # oT = sum_r WoPad_r.T @ cxs_r  (+bo on eviction)
                if "attn_tail" not in ablate:
                    ot_p = ps.tile([48, 128], f32, tag="late", name="ot_p")
                    nc.tensor.matmul(
                        ot_p[:], wopa_s[:], cxs[0][:], start=True, stop=False
                    )
                    nc.tensor.matmul(
                        ot_p[:], wopb_s[:], cxs[1][:], start=False, stop=True
                    )
                    ot_s = sm_sb.tile([48, 128], f32r, tag="ots", name="ot_s")
                    nc.scalar.activation(
                        out=ot_s[:], in_=ot_p[:], func=AF.Identity, bias=bo_s[:]
                    )

                # y1 = x + o via two transposes into one PSUM accum group
                y1w = y1_p[:, wi, :]
                nc.tensor.matmul(
                    y1w,
                    xsl[0:48, w0 : w0 + T],
                    idf[0:48, 0:48],
                    is_transpose=True,
                    start=True,
                    stop=False,
                )
                nc.tensor.matmul(
                    y1w,
                    ot_s[:],
                    idf[0:48, 0:48],
                    is_transpose=True,
                    start=False,
                    stop=True,
                )


            # batched LN1 over the whole chunk (PSUM input; every ln_batch
            # op reads it at most once per instruction)
            h1_s = ln_batch(y1_p[:].bitcast(f32), ln_s[:, 0, :], ln_s[:, 1, :],
                            "ln1", ch)
            for wi in range(CHUNK):
                w0 = wi * T
                h1t_p = ps.tile([48, 128], f32r, tag="late", name="h1t_p")
                nc.tensor.transpose(h1t_p[:], h1_s[:, wi, :], idf[:])
                nc.scalar.copy(out=h1t_s[:, w0 : w0 + T], in_=h1t_p[:])

            fr_s = []
            for half in range(2):
                ft_p = ps.tile([128, NTK], f32, tag=("late" if half == 0 else "misc"), name=f"ft{half}")
                nc.tensor.matmul(
                    ft_p[:], w_tiles["w1"][:, 128 * half : 128 * half + 128],
                    h1t_s[:],
                )
                fr = ff_sb.tile([128, NTK], f32r, tag=f"fr{half}", name=f"fr{half}")
                if half == 0:
                    nc.scalar.activation(
                        out=fr[:], in_=ft_p[:], func=AF.Relu,
                        bias=b1_s[:, half : half + 1],
                    )
                else:
                    nc.vector.tensor_scalar(
                        out=fr[:], in0=ft_p[:],
                        scalar1=b1_s[:, half : half + 1], scalar2=0.0,
                        op0=ALU.add, op1=ALU.max,
                    )
                fr_s.append(fr)
            y2t_p = ps.tile([48, NTK], f32, tag="y1", name="y2t_p")
            nc.tensor.matmul(
                y2t_p[:], w2_s[:, 0, :], fr_s[0][:], start=True, stop=False
            )
            nc.tensor.matmul(
                y2t_p[:], w2_s[:, 1, :], fr_s[1][:], start=False, stop=True
            )
            y2t_s = ff_sb.tile([48, NTK], f32r, tag="y2ts", name="y2t_s")
            nc.scalar.activation(
                out=y2t_s[:], in_=y2t_p[:], func=AF.Identity, bias=b2_s[:]
            )

            y2_p = ps.tile([128, CHUNK, 48], f32r, tag="y1", name="y2_p")
            for wi in range(CHUNK):
                w0 = wi * T
                nc.tensor.transpose(
                    y2_p[:, wi, :], y2t_s[:, w0 : w0 + T], idf[0:48, 0:48]
                )
            y3 = out_sb.tile([128, CHUNK, 48], f32, tag="y3", name="y3")
            nc.vector.tensor_add(
                y3[:], y2_p[:].bitcast(f32), h1_s[:].bitcast(f32)
            )
            h2 = ln_batch(y3[:], ln_s[:, 2, :], ln_s[:, 3, :], "ln2", ch, out_dtype=f32)
            if "dma" in ablate:
                continue
            nc.sync.dma_start(
                out=out.ap()[ch * CHUNK : (ch + 1) * CHUNK, :, :].rearrange(
                    "w t c -> t w c"
                ),
                in_=h2[:].bitcast(f32),
            )
        if rep_cm is not None:
            rep_cm.__exit__(None, None, None)

    nc.compile()
    return nc


def _prep_host(voxel_features, voxel_coords, Wq, bq, Wk, bk, Wv, bv, Wo, bo,
               ln1_g, ln1_b, W1, b1, W2, b2, ln2_g, ln2_b, wpc=WPC,
               ncores=NCORES):
    f32 = np.float32
    vc = np.asarray(voxel_coords)
    b, z, y, x = vc[:, 0], vc[:, 1], vc[:, 2], vc[:, 3]
    win = ((b * (GZ // WZ) + z // WZ) * (GY // WY) + y // WY) * (GX // WX) + x // WX
    slot = (z % WZ) * (WY * WX) + (y % WY) * WX + (x % WX)
    win = np.asarray(win, np.int64)
    slot = np.asarray(slot, np.int64)

    nwp = ncores * wpc
    xta = np.zeros((nwp, 50, T), f32)
    xta[:, 48, :] = 1.0
    xta[win, :48, slot] = np.asarray(voxel_features, f32)
    mask = np.full((nwp, T), MASKVAL, f32)
    occupied = np.zeros(nwp, bool)
    occupied[win] = True
    mask[~occupied] = 0.0
    mask[win, slot] = 0.0
    xta[:, 49, :] = mask

    s = f32(1.0 / np.sqrt(HD))
    Wq_s = np.asarray(Wq, f32) * s
    bq_s = np.asarray(bq, f32) * s
    bf = ml_dtypes.bfloat16

    def qk_pack(W, bvec, mask_lane):
        A = np.zeros((2, 50, 128), f32)
        for h in range(8):
            half, i = divmod(h, 4)
            A[half, :48, 32 * i : 32 * i + 6] = W[:, 6 * h : 6 * h + 6]
            A[half, 48, 32 * i : 32 * i + 6] = bvec[6 * h : 6 * h + 6]
            A[half, 49 if mask_lane else 48, 32 * i + 6] = 1.0
        return A[0], A[1]

    wqa_a, wqb_a = qk_pack(Wq_s, bq_s, mask_lane=False)
    wka_a, wkb_a = qk_pack(np.asarray(Wk, f32), np.asarray(bk, f32),
                           mask_lane=True)
    wv_a = np.zeros((50, 48), f32)
    wv_a[:48] = np.asarray(Wv, f32)
    wv_a[48] = np.asarray(bv, f32)
    wop = np.zeros((2, 128, 48), f32)
    for h in range(8):
        r, j = divmod(h, 4)
        wop[r, 32 * j : 32 * j + 6, :] = np.asarray(Wo, f32)[6 * h : 6 * h + 6, :]
    w1_a = np.ascontiguousarray(np.asarray(W1, f32))
    b1c_a = np.stack([np.asarray(b1, f32)[:128], np.asarray(b1, f32)[128:]], 1)
    b2c_a = np.asarray(b2, f32).reshape(48, 1)
    ln_a = np.stack([ln1_g, ln1_b, ln2_g, ln2_b]).astype(f32)

    weights = dict(
        wqa=wqa_a, wqb=wqb_a, wka=wka_a, wkb=wkb_a, wv=wv_a,
        wopa=wop[0].astype(bf), wopb=wop[1].astype(bf),
        boc=np.asarray(bo, f32).reshape(48, 1),
        w1=w1_a, w2=np.ascontiguousarray(np.asarray(W2, f32)),
        b1c=np.ascontiguousarray(b1c_a), b2c=b2c_a,
        lnc=np.ascontiguousarray(ln_a),
    )
    in_maps = []
    for c in range(ncores):
        m = dict(weights)
        sh = xta[c * wpc : (c + 1) * wpc]  # [wpc, 50, T]
        m["xta"] = np.ascontiguousarray(
            sh.transpose(1, 0, 2).reshape(50, wpc * T)
        )
        in_maps.append(m)
    return in_maps, win, slot


def kernel(**inputs):
    key = ("full", WPC)
    if key not in _CACHE:
        _CACHE[key] = _build_bass(WPC)
    nc = _CACHE[key]
    in_maps, win, slot = _prep_host(**inputs)
    from concourse import bass_utils

    r = bass_utils.run_bass_kernel_spmd(
        nc, in_maps, core_ids=list(range(NCORES))
    )
    full = np.concatenate([r.results[c]["out"] for c in range(NCORES)], 0)
    return full[win, slot].astype(np.float32)


# revision 36
# speedup vs baseline: 1.6109x; 1.4270x over previous
"""Windowed sparse point-transformer layer on 8 Trainium2 NeuronCores.

Strategy (spec sharding_hint): windows are independent, so the host
scatters voxels into dense per-window buffers (the "all-to-all keyed by
window id" done as a host-side relayout), shards 900->928 windows across
8 cores (116 each), and each core runs a dense windowed attention+FFN
Bass kernel. Weights are replicated. The host gathers per-window outputs
back to the sparse voxel list.

Device kernel per window (T=128 slots, C=48 channels, 8 heads x 6 dim):
  - q/k projections produce CHANNEL-major qT/kT with heads padded onto
    32-partition strips so the 8 score matmuls use PE row tiling
    (K=8 per head: 6 data lanes + a ones/mask lane that applies the
    -1e5 padding mask during the matmul itself; bias is folded in via
    the input's ones row).
  - softmax (q-major): exp on ACT with fused row-sum accumulation,
    reciprocal + per-head normalize on DVE (bf16), attn transposed on
    PE, ctx matmuls against bf16 v accumulate fp32 in PSUM.
  - residual 1: x.T is PE-transposed straight into the same PSUM
    accumulation group as the output projection (ctx @ Wo + bo).
  - FFN batched across a 4-window chunk (N=512) in float32r so the PE
    runs at full rate; b1/b2 ride the activation bias port during PSUM
    eviction. LayerNorms run token-major on DVE/ACT (bn_stats/bn_aggr).

PSUM is 8 banks; tile tags are assigned so the live set maps onto 8
bank-sized slots (s0-s3 shared by projections/scores/FFN, at, misc,
cx, y1).
"""

import numpy as np
import ml_dtypes
from contextlib import ExitStack

GX, GY, GZ = 120, 120, 8
WX, WY, WZ = 8, 8, 2
T = 128
C = 48
H = 8
HD = 6
FF = 256
N = 80000
NW = (GX // WX) * (GY // WY) * (GZ // WZ)  # 900
NCORES = 8
WPC = 116  # windows per core (900 padded to 928 = 8*116)
CHUNK = 4  # windows per projection/FFN batch (N=512 tokens)
MASKVAL = -1e5

_CACHE = {}


def _build_bass(wpc, reps=1, ablate=()):
    import concourse.bass as bass
    import concourse.tile as tile
    import concourse.mybir as mybir
    from concourse import bacc
    from concourse.masks import make_identity

    f32 = mybir.dt.float32
    f32r = mybir.dt.float32r
    bf16 = mybir.dt.bfloat16
    AF = mybir.ActivationFunctionType
    ALU = mybir.AluOpType

    nc = bacc.Bacc("TRN2", target_bir_lowering=False, debug=False)
    ntok = wpc * T

    xta = nc.dram_tensor("xta", [50, ntok], f32r, kind="ExternalInput")
    wqa = nc.dram_tensor("wqa", [50, 128], f32r, kind="ExternalInput")
    wqb = nc.dram_tensor("wqb", [50, 128], f32r, kind="ExternalInput")
    wka = nc.dram_tensor("wka", [50, 128], f32r, kind="ExternalInput")
    wkb = nc.dram_tensor("wkb", [50, 128], f32r, kind="ExternalInput")
    wv = nc.dram_tensor("wv", [50, 48], f32r, kind="ExternalInput")
    wopa = nc.dram_tensor("wopa", [128, 48], bf16, kind="ExternalInput")
    wopb = nc.dram_tensor("wopb", [128, 48], bf16, kind="ExternalInput")
    boc = nc.dram_tensor("boc", [48, 1], f32, kind="ExternalInput")
    w1 = nc.dram_tensor("w1", [48, 256], f32r, kind="ExternalInput")
    w2 = nc.dram_tensor("w2", [256, 48], f32r, kind="ExternalInput")
    b1c = nc.dram_tensor("b1c", [128, 2], f32, kind="ExternalInput")
    b2c = nc.dram_tensor("b2c", [48, 1], f32, kind="ExternalInput")
    lnc = nc.dram_tensor("lnc", [4, 48], f32, kind="ExternalInput")
    out = nc.dram_tensor("out", [wpc, T, C], f32, kind="ExternalOutput")

    nchunk = wpc // CHUNK
    assert wpc % CHUNK == 0

    def _bn(col, n):
        return bass.AP(tensor=col.tensor, offset=col.offset,
                       ap=[col.ap[0], [0, n]])

    def _b(ap3, n=None):
        # broadcast a [128, k] AP along a new trailing free dim of size 48
        return bass.AP(tensor=ap3.tensor, offset=ap3.offset,
                       ap=list(ap3.ap) + [[0, 48]])

    def ln_apply(pool, y_ap, g_ap, b_ap, eps_s, out_dtype, tagp):
        """LayerNorm over the free dim (C=48) of token-major y_ap."""
        mv = pool.tile([128, 2], f32, tag=f"{tagp}mv", name=f"{tagp}mv")
        st6 = pool.tile([128, 6], f32, tag=f"{tagp}st", name=f"{tagp}st")
        nc.vector.bn_stats(out=st6[:], in_=y_ap)
        nc.vector.bn_aggr(out=mv[:], in_=st6[:])
        rstd = pool.tile([128, 1], f32, tag=f"{tagp}rs", name=f"{tagp}rs")
        nc.scalar.activation(
            out=rstd[:], in_=mv[:, 1:2], func=AF.Sqrt, bias=eps_s[:]
        )
        nc.vector.reciprocal(rstd[:], rstd[:])
        yc = pool.tile([128, 48], f32, tag=f"{tagp}yc", name=f"{tagp}yc")
        nc.vector.tensor_scalar_sub(yc[:], y_ap, mv[:, 0:1])
        ho = pool.tile([128, 48], out_dtype, tag=f"{tagp}ho", name=f"{tagp}ho")
        nc.vector.scalar_tensor_tensor(
            out=ho[:], in0=yc[:], scalar=rstd[:], in1=g_ap,
            op0=ALU.mult, op1=ALU.mult,
        )
        nc.vector.tensor_add(ho[:], ho[:], b_ap)
        return ho

    with tile.TileContext(nc) as tc, ExitStack() as ctx:
        singles = ctx.enter_context(tc.tile_pool(name="singles", bufs=1))

        xta_s = singles.tile([50, ntok], f32r)
        nc.sync.dma_start(out=xta_s[:], in_=xta.ap())
        w_tiles = {}
        for nm, hnd, shp in (
            ("wqa", wqa, [50, 128]), ("wqb", wqb, [50, 128]),
            ("wka", wka, [50, 128]), ("wkb", wkb, [50, 128]),
            ("wv", wv, [50, 48]), ("w1", w1, [48, 256]),
        ):
            t = singles.tile(shp, f32r, name=nm)
            nc.sync.dma_start(out=t[:], in_=hnd.ap())
            w_tiles[nm] = t
        wopa_s = singles.tile([128, 48], bf16)
        nc.sync.dma_start(out=wopa_s[:], in_=wopa.ap())
        wopb_s = singles.tile([128, 48], bf16)
        nc.sync.dma_start(out=wopb_s[:], in_=wopb.ap())
        bo_s = singles.tile([48, 1], f32)
        nc.sync.dma_start(out=bo_s[:], in_=boc.ap())
        w2_s = singles.tile([128, 2, 48], f32r)
        nc.sync.dma_start(
            out=w2_s[:], in_=w2.ap().rearrange("(two p) n -> p two n", two=2)
        )
        b1_s = singles.tile([128, 2], f32)
        nc.sync.dma_start(out=b1_s[:], in_=b1c.ap())
        b2_s = singles.tile([48, 1], f32)
        nc.sync.dma_start(out=b2_s[:], in_=b2c.ap())
        ln_s = singles.tile([128, 4, 48], f32)
        lnap = lnc.ap()
        ln_bcast = bass.AP(
            tensor=lnap.tensor, offset=lnap.offset, ap=[[0, 128]] + list(lnap.ap)
        )
        nc.sync.dma_start(out=ln_s[:], in_=ln_bcast)
        eps_s = singles.tile([128, 1], f32)
        nc.vector.memset(eps_s[:], 1e-5)
        idb = singles.tile([128, 128], bf16)
        make_identity(nc, idb[:])
        idtmp = singles.tile([128, 128], f32)
        make_identity(nc, idtmp[:])
        idf = singles.tile([128, 128], f32r)
        nc.vector.tensor_copy(out=idf[:], in_=idtmp[:])

        ps = ctx.enter_context(tc.tile_pool(name="ps", bufs=1, space="PSUM"))
        pss = ctx.enter_context(tc.tile_pool(name="pss", bufs=1, space="PSUM"))
        # ctx accumulators: heads live at partitions 32j..32j+6; the gap rows
        # must be exactly zero (o-projection multiplies them by zero weights,
        # and stale PSUM could be non-finite), so zero them once up front.
        cxp_t = pss.tile([128, 2, 128], f32, name="cxp_t")
        nc.vector.memset(cxp_t[:], 0.0)
        cxp = [cxp_t[:, 0, :], cxp_t[:, 1, :]]
        qk_sb = ctx.enter_context(tc.tile_pool(name="qk_sb", bufs=3))
        sm_sb = ctx.enter_context(tc.tile_pool(name="sm_sb", bufs=3))
        ln_sb = ctx.enter_context(tc.tile_pool(name="ln_sb", bufs=3))
        ff_sb = ctx.enter_context(tc.tile_pool(name="ff_sb", bufs=3))
        out_sb = ctx.enter_context(tc.tile_pool(name="out_sb", bufs=3))

        AX = mybir.AxisListType

        def ln_batch(y_ap, g_ap, b_ap, pfx, ch, out_dtype=None):
            """Chunk-batched LayerNorm: y_ap is [128, CHUNK, 48] (PSUM ok)."""
            od = out_dtype if out_dtype is not None else f32r
            P = ln_sb
            ssum = P.tile([128, CHUNK], f32, tag=f"{pfx}sum", name=f"{pfx}sum")
            nc.vector.tensor_reduce(out=ssum[:], in_=y_ap, axis=AX.X, op=ALU.add)
            sq = P.tile([128, CHUNK, 48], f32, tag=f"{pfx}sq", name=f"{pfx}sq")
            nc.scalar.square(out=sq[:], in_=y_ap)
            sqs = P.tile([128, CHUNK], f32, tag=f"{pfx}sqs", name=f"{pfx}sqs")
            nc.vector.tensor_reduce(out=sqs[:], in_=sq[:], axis=AX.X, op=ALU.add)
            mean = P.tile([128, CHUNK], f32, tag=f"{pfx}mn", name=f"{pfx}mn")
            nc.vector.tensor_scalar_mul(mean[:], ssum[:], 1.0 / 48.0)
            msq = P.tile([128, CHUNK], f32, tag=f"{pfx}msq", name=f"{pfx}msq")
            nc.vector.tensor_mul(msq[:], mean[:], mean[:])
            var = P.tile([128, CHUNK], f32, tag=f"{pfx}var", name=f"{pfx}var")
            nc.vector.scalar_tensor_tensor(
                out=var[:], in0=sqs[:], scalar=1.0 / 48.0, in1=msq[:],
                op0=ALU.mult, op1=ALU.subtract,
            )
            rstd = P.tile([128, CHUNK], f32, tag=f"{pfx}rs", name=f"{pfx}rs")
            nc.scalar.activation(
                out=rstd[:], in_=var[:], func=AF.Sqrt, bias=eps_s[:]
            )
            nc.vector.reciprocal(rstd[:], rstd[:])
            t = P.tile([128, CHUNK, 48], f32, tag=f"{pfx}t", name=f"{pfx}t")
            nc.vector.tensor_sub(t[:], y_ap, _b(mean[:]))
            nc.vector.tensor_mul(t[:], t[:], _b(rstd[:]))
            gv = bass.AP(tensor=g_ap.tensor, offset=g_ap.offset,
                         ap=[g_ap.ap[0], [0, CHUNK], g_ap.ap[1]])
            bv = bass.AP(tensor=b_ap.tensor, offset=b_ap.offset,
                         ap=[b_ap.ap[0], [0, CHUNK], b_ap.ap[1]])
            h = P.tile([128, CHUNK, 48], od, tag=f"{pfx}h", name=f"{pfx}h")
            nc.vector.tensor_mul(h[:], t[:], gv)
            nc.vector.tensor_add(h[:], h[:], bv)
            return h

        rep_cm = tc.For_i(0, reps, 1) if reps > 1 else None
        if rep_cm is not None:
            rep_cm.__enter__()
        for ch in range(nchunk):
            t0 = ch * CHUNK * T
            NTK = CHUNK * T
            xsl = xta_s[:, t0 : t0 + NTK]

            projs = {}
            for nm, wt, tag in (
                ("qa", "wqa", "s0"), ("qb", "wqb", "s1"),
                ("ka", "wka", "s2"), ("kb", "wkb", "s3"),
            ):
                p = ps.tile([128, NTK], f32, tag=tag, name=f"p{nm}")
                nc.tensor.matmul(p[:], w_tiles[wt][:], xsl)
                projs[nm] = p
            vt_p = ps.tile([48, NTK], f32, tag="misc", name="vt_p")
            nc.tensor.matmul(vt_p[:], w_tiles["wv"][:], xsl)

            qkev = {}
            for i, nm in enumerate(("qa", "qb", "ka", "kb")):
                t = qk_sb.tile([128, NTK], bf16, tag=nm, name=f"s{nm}")
                if i % 2 == 0:
                    nc.vector.tensor_copy(out=t[:], in_=projs[nm][:])
                else:
                    nc.scalar.copy(out=t[:], in_=projs[nm][:])
                qkev[nm] = t
            vt_s = qk_sb.tile([48, NTK], bf16, tag="vt", name="vt_s")
            nc.scalar.copy(out=vt_s[:], in_=vt_p[:])

            h1t_s = ff_sb.tile([48, NTK], f32r, tag="h1t", name="h1t_s")

            y1_p = ps.tile([128, CHUNK, 48], f32r, tag="y1", name="y1_p")
            for wi in range(CHUNK):
                w0 = wi * T

                v_ps = ps.tile([128, 48], bf16, tag="misc", name="v_ps")
                nc.tensor.transpose(v_ps[:], vt_s[:, w0 : w0 + T], idb[0:48, 0:48])
                v_s = sm_sb.tile([128, 48], bf16, tag="v", name="v_s")
                nc.vector.tensor_copy(out=v_s[:], in_=v_ps[:])

                scp = [
                    ps.tile([128, 2, 128], f32, tag=f"s{i}", name=f"scp{i}")
                    for i in range(4)
                ]
                if "scores" not in ablate:
                    for rnd, (qs, ks) in enumerate(
                        ((qkev["qa"], qkev["ka"]), (qkev["qb"], qkev["kb"]))
                    ):
                        for i in range(4):
                            nc.tensor.matmul(
                                scp[i][:, rnd, :],
                                qs[32 * i : 32 * i + 8, w0 : w0 + T],
                                ks[32 * i : 32 * i + 8, w0 : w0 + T],
                                tile_position=(32 * i, 0),
                            )

                # attn layout: [128, strip i, round r, 128]; head h = 4*r + i
                attn_s = sm_sb.tile([128, 4, 2, 128], bf16, tag="attn", name="attn_s")
                for i in range(4 if "scores" not in ablate else 0):
                    nc.scalar.activation(
                        out=attn_s[:, i, :, :], in_=scp[i][:, :, :], func=AF.Exp
                    )
                if "attn_tail" in ablate:
                    ot_s = sm_sb.tile([48, 128], f32r, tag="ots", name="ot_s")
                    nc.scalar.copy(out=ot_s[:], in_=xsl[0:48, w0 : w0 + T])
                rsum = sm_sb.tile([128, 4, 2], f32, tag="rsum", name="rsum")
                rrec = sm_sb.tile([128, 4, 2], bf16, tag="rrec", name="rrec")
                for j in range(2):
                    sl = slice(2 * j, 2 * j + 2)
                    nc.vector.tensor_reduce(
                        out=rsum[:, sl, :], in_=attn_s[:, sl, :, :], axis=AX.X,
                        op=ALU.add,
                    )
                    with nc.allow_low_precision(reason="softmax denom in bf16"):
                        nc.vector.reciprocal(rrec[:, sl, :], rsum[:, sl, :])
                    rj = rrec[:, sl, :]
                    rrec_b = bass.AP(
                        tensor=rj.tensor, offset=rj.offset,
                        ap=list(rj.ap) + [[0, 128]],
                    )
                    nc.gpsimd.tensor_mul(
                        attn_s[:, sl, :, :], attn_s[:, sl, :, :], rrec_b
                    )

                atp = ps.tile([128, 8, 128], bf16, tag="misc", name="atp")
                for h in range(8 if "attn_tail" not in ablate else 0):
                    nc.tensor.transpose(
                        atp[:, h, :], attn_s[:, h % 4, h // 4, :], idb[:]
                    )
                atn_s = sm_sb.tile([128, 8, 128], bf16, tag="atn", name="atn_s")
                if "attn_tail" not in ablate:
                    nc.vector.tensor_copy(out=atn_s[:, 0:4, :], in_=atp[:, 0:4, :])
                    nc.scalar.copy(out=atn_s[:, 4:8, :], in_=atp[:, 4:8, :])

                # ctx col-tiled: round r holds heads 4r+j at partitions 32j
                for h in range(8 if "attn_tail" not in ablate else 0):
                    r, j = divmod(h, 4)
                    nc.tensor.matmul(
                        cxp[r][32 * j : 32 * j + 6, :],
                        v_s[:, 6 * h : 6 * h + 6],
                        atn_s[:, h, :],
                        tile_position=(0, 32 * j),
                    )
                cxs = []
                for r in range(2 if "attn_tail" not in ablate else 0):
                    t = sm_sb.tile([128, 128], bf16, tag=f"cxs{r}", name=f"cxs{r}")
                    if r == 0:
                        nc.vector.tensor_copy(out=t[:], in_=cxp[r])
                    else:
                        nc.scalar.copy(out=t[:], in_=cxp[r])
                    cxs.append(t)

                # oT = sum_r WoPad_r.T @ cxs_r  (+bo on eviction)
                if "attn_tail" not in ablate:
                    ot_p = ps.tile([48, 128], f32, tag="late", name="ot_p")
                    nc.tensor.matmul(
                        ot_p[:], wopa_s[:], cxs[0][:], start=True, stop=False
                    )
                    nc.tensor.matmul(
                        ot_p[:], wopb_s[:], cxs[1][:], start=False, stop=True
                    )
                    ot_s = sm_sb.tile([48, 128], f32r, tag="ots", name="ot_s")
                    nc.scalar.activation(
                        out=ot_s[:], in_=ot_p[:], func=AF.Identity, bias=bo_s[:]
                    )

                # y1 = x + o via two transposes into one PSUM accum group
                y1w = y1_p[:, wi, :]
                nc.tensor.matmul(
                    y1w,
                    xsl[0:48, w0 : w0 + T],
                    idf[0:48, 0:48],
                    is_transpose=True,
                    start=True,
                    stop=False,
                )
                nc.tensor.matmul(
                    y1w,
                    ot_s[:],
                    idf[0:48, 0:48],
                    is_transpose=True,
                    start=False,
                    stop=True,
                )


            # batched LN1 over the whole chunk (PSUM input; every ln_batch
            # op reads it at most once per instruction)
            h1_s = ln_batch(y1_p[:].bitcast(f32), ln_s[:, 0, :], ln_s[:, 1, :],
                            "ln1", ch)
            for wi in range(CHUNK):
                w0 = wi * T
                h1t_p = ps.tile([48, 128], f32r, tag="late", name="h1t_p")
                nc.tensor.transpose(h1t_p[:], h1_s[:, wi, :], idf[:])
                nc.scalar.copy(out=h1t_s[:, w0 : w0 + T], in_=h1t_p[:])

            fr_s = []
            for half in range(2):
                ft_p = ps.tile([128, NTK], f32, tag=("late" if half == 0 else "misc"), name=f"ft{half}")
                nc.tensor.matmul(
                    ft_p[:], w_tiles["w1"][:, 128 * half : 128 * half + 128],
                    h1t_s[:],
                )
                fr = ff_sb.tile([128, NTK], f32r, tag=f"fr{half}", name=f"fr{half}")
                if half == 0:
                    nc.scalar.activation(
                        out=fr[:], in_=ft_p[:], func=AF.Relu,
                        bias=b1_s[:, half : half + 1],
                    )
                else:
                    nc.vector.tensor_scalar(
                        out=fr[:], in0=ft_p[:],
                        scalar1=b1_s[:, half : half + 1], scalar2=0.0,
                        op0=ALU.add, op1=ALU.max,
                    )
                fr_s.append(fr)
            y2t_p = ps.tile([48, NTK], f32, tag="y1", name="y2t_p")
            nc.tensor.matmul(
                y2t_p[:], w2_s[:, 0, :], fr_s[0][:], start=True, stop=False
            )
            nc.tensor.matmul(
                y2t_p[:], w2_s[:, 1, :], fr_s[1][:], start=False, stop=True
            )
            y2t_s = ff_sb.tile([48, NTK], f32r, tag="y2ts", name="y2t_s")
            nc.scalar.activation(
                out=y2t_s[:], in_=y2t_p[:], func=AF.Identity, bias=b2_s[:]
            )

            y2_p = ps.tile([128, CHUNK, 48], f32r, tag="y1", name="y2_p")
            for wi in range(CHUNK):
                w0 = wi * T
                nc.tensor.transpose(
                    y2_p[:, wi, :], y2t_s[:, w0 : w0 + T], idf[0:48, 0:48]
                )
            y3 = out_sb.tile([128, CHUNK, 48], f32, tag="y3", name="y3")
            nc.vector.tensor_add(
                y3[:], y2_p[:].bitcast(f32), h1_s[:].bitcast(f32)
            )
            h2 = ln_batch(y3[:], ln_s[:, 2, :], ln_s[:, 3, :], "ln2", ch, out_dtype=f32)
            if "dma" in ablate:
                continue
            nc.sync.dma_start(
                out=out.ap()[ch * CHUNK : (ch + 1) * CHUNK, :, :].rearrange(
                    "w t c -> t w c"
                ),
                in_=h2[:].bitcast(f32),
            )
        if rep_cm is not None:
            rep_cm.__exit__(None, None, None)

    nc.compile()
    return nc


def _prep_host(voxel_features, voxel_coords, Wq, bq, Wk, bk, Wv, bv, Wo, bo,
               ln1_g, ln1_b, W1, b1, W2, b2, ln2_g, ln2_b, wpc=WPC,
               ncores=NCORES):
    f32 = np.float32
    vc = np.asarray(voxel_coords)
    b, z, y, x = vc[:, 0], vc[:, 1], vc[:, 2], vc[:, 3]
    win = ((b * (GZ // WZ) + z // WZ) * (GY // WY) + y // WY) * (GX // WX) + x // WX
    slot = (z % WZ) * (WY * WX) + (y % WY) * WX + (x % WX)
    win = np.asarray(win, np.int64)
    slot = np.asarray(slot, np.int64)

    nwp = ncores * wpc
    xta = np.zeros((nwp, 50, T), f32)
    xta[:, 48, :] = 1.0
    xta[win, :48, slot] = np.asarray(voxel_features, f32)
    mask = np.full((nwp, T), MASKVAL, f32)
    occupied = np.zeros(nwp, bool)
    occupied[win] = True
    mask[~occupied] = 0.0
    mask[win, slot] = 0.0
    xta[:, 49, :] = mask

    s = f32(1.0 / np.sqrt(HD))
    Wq_s = np.asarray(Wq, f32) * s
    bq_s = np.asarray(bq, f32) * s
    bf = ml_dtypes.bfloat16

    def qk_pack(W, bvec, mask_lane):
        A = np.zeros((2, 50, 128), f32)
        for h in range(8):
            half, i = divmod(h, 4)
            A[half, :48, 32 * i : 32 * i + 6] = W[:, 6 * h : 6 * h + 6]
            A[half, 48, 32 * i : 32 * i + 6] = bvec[6 * h : 6 * h + 6]
            A[half, 49 if mask_lane else 48, 32 * i + 6] = 1.0
        return A[0], A[1]

    wqa_a, wqb_a = qk_pack(Wq_s, bq_s, mask_lane=False)
    wka_a, wkb_a = qk_pack(np.asarray(Wk, f32), np.asarray(bk, f32),
                           mask_lane=True)
    wv_a = np.zeros((50, 48), f32)
    wv_a[:48] = np.asarray(Wv, f32)
    wv_a[48] = np.asarray(bv, f32)
    wop = np.zeros((2, 128, 48), f32)
    for h in range(8):
        r, j = divmod(h, 4)
        wop[r, 32 * j : 32 * j + 6, :] = np.asarray(Wo, f32)[6 * h : 6 * h + 6, :]
    w1_a = np.ascontiguousarray(np.asarray(W1, f32))
    b1c_a = np.stack([np.asarray(b1, f32)[:128], np.asarray(b1, f32)[128:]], 1)
    b2c_a = np.asarray(b2, f32).reshape(48, 1)
    ln_a = np.stack([ln1_g, ln1_b, ln2_g, ln2_b]).astype(f32)

    weights = dict(
        wqa=wqa_a, wqb=wqb_a, wka=wka_a, wkb=wkb_a, wv=wv_a,
        wopa=wop[0].astype(bf), wopb=wop[1].astype(bf),
        boc=np.asarray(bo, f32).reshape(48, 1),
        w1=w1_a, w2=np.ascontiguousarray(np.asarray(W2, f32)),
        b1c=np.ascontiguousarray(b1c_a), b2c=b2c_a,
        lnc=np.ascontiguousarray(ln_a),
    )
    in_maps = []
    for c in range(ncores):
        m = dict(weights)
        sh = xta[c * wpc : (c + 1) * wpc]  # [wpc, 50, T]
        m["xta"] = np.ascontiguousarray(
            sh.transpose(1, 0, 2).reshape(50, wpc * T)
        )
        in_maps.append(m)
    return in_maps, win, slot


def kernel(**inputs):
    key = ("full", WPC)
    if key not in _CACHE:
        _CACHE[key] = _build_bass(WPC)
    nc = _CACHE[key]
    in_maps, win, slot = _prep_host(**inputs)
    from concourse import bass_utils

    r = bass_utils.run_bass_kernel_spmd(
        nc, in_maps, core_ids=list(range(NCORES))
    )
    full = np.concatenate([r.results[c]["out"] for c in range(NCORES)], 0)
    return full[win, slot].astype(np.float32)
